# revision 23
# baseline (speedup 1.0000x reference)
"""Deformable Conv2d (B=4, Cin=64, Cout=128, H=W=128, K=3) on 8 trn2 cores.

Sharding: data-parallel over (batch, H-half): core s -> image s//2,
rows [64*(s%2), +64). All FLOPs on device:
  - offset/modulator 3x3 convs on PE (pos-major out via x-as-lhsT)
  - bilinear corner weights + gather indices on DVE/ACT
  - 4-corner gather via SWDGE dma_gather (512B/descriptor, bf16)
  - corner combine: one broadcast tensor_tensor multiply per tap-group
  - corner-sum + transpose via PE transpose-accumulate into PSUM
  - 576->128 einsum on PE (bf16, f32 PSUM)
Host side: input layout prep (padded shards, row-pair-duplicated gather
source, weight reordering, constant tables) and output reassembly.
"""

import numpy as np
import ml_dtypes

import concourse.bass as bass
import concourse.bacc as bacc
import concourse.mybir as mybir
from concourse.tile import TileContext
from concourse import library_config

F32 = mybir.dt.float32
BF16 = mybir.dt.bfloat16
I16 = mybir.dt.int16
I32 = mybir.dt.int32
ALU = mybir.AluOpType
_FLOOR_BIAS = -0.5  # HW float->int cast rounds; sim truncates (use 0.0)
ACTF = mybir.ActivationFunctionType

B, Cin, Cout, H, W = 4, 64, 128, 128, 128
Hs = 64                      # rows per shard
PADY = PADX = 4
Hp, Wp = 73, 136             # X2 padded dims
CONV_H, CONV_W = Hs + 2, W + 2   # 66 x 130 conv input (pad 1)
CONV_FLAT = CONV_H * CONV_W      # 8580
ELEM = 256                   # gather element: (xc2, c64, yc2) bf16 = 512B
HB = 16                      # rows per processing chunk
NCH = Hs // HB               # chunks per shard
NPOS_HB = HB * W             # positions per chunk
JP = NPOS_HB // 16           # wrapped idx free-pitch per tap
TAPS = 9
# tap groups for conv / gather / combine stages
# (pair groups use the +1-shifted second half of the x tile; K=128)
GROUPS = [(0, 1), (3, 4), (6, 7), (2,), (5,), (8,)]


def _conv_off(k, h):
    ky, kx = divmod(k, 3)
    return (h + ky) * CONV_W + kx


def build_nc():
    nc = bacc.Bacc("TRN2", num_swdge_queues=4)

    xab = nc.dram_tensor("xab", [128, CONV_FLAT], BF16, kind="ExternalInput")
    # gather source: point rows of 128 (c, yc); an element spans two
    # consecutive points (xc) = 256 values, so rows overlap (elem_step=128)
    x2 = nc.dram_tensor("x2", [Hp * Wp + 1, 128], BF16, kind="ExternalInput")
    ck = nc.dram_tensor("ck", [128, 27], F32, kind="ExternalInput")
    chv = nc.dram_tensor("chv", [128, Hs], F32, kind="ExternalInput")
    pvec = nc.dram_tensor("pvec", [128, 1], F32, kind="ExternalInput")
    wconv = nc.dram_tensor("wconv", [128, 6, 27], BF16, kind="ExternalInput")
    # einsum weights: chunk per tap, rows = (c, yc) (yc-duplicated)
    wmain = nc.dram_tensor("wmain", [128, TAPS, 128], BF16,
                           kind="ExternalInput")
    ident = nc.dram_tensor("ident", [128, 128], BF16, kind="ExternalInput")
    out = nc.dram_tensor("out", [128, Hs * W], F32, kind="ExternalOutput")
    # scratch layout: addr = p*(TAPS*HB) + k*HB + h  (per-partition contiguous)
    idxs_dram = nc.dram_tensor("idxs_scratch", [NCH, 128 * TAPS * HB], I16,
                               kind="Internal")

    with TileContext(nc) as tc:
        with tc.tile_pool(name="static", bufs=1) as static, \
             tc.tile_pool(name="offp", bufs=2) as offp, \
             tc.tile_pool(name="fld", bufs=2) as fld, \
             tc.tile_pool(name="idxp", bufs=3) as idxp, \
             tc.tile_pool(name="vp", bufs=4) as vp, \
             tc.tile_pool(name="stp", bufs=2) as stp, \
             tc.tile_pool(name="outp", bufs=2) as outp, \
             tc.tile_pool(name="pconv", bufs=2, space="PSUM") as pconv, \
             tc.tile_pool(name="ptac", bufs=2, space="PSUM") as ptac, \
             tc.tile_pool(name="pout", bufs=1, space="PSUM") as pout:

            nc.gpsimd.load_library(library_config.mlp)
            gather_seq = [0]

            # ---- static tiles ----
            t_xab = static.tile([128, CONV_FLAT], BF16)
            nc.sync.dma_start(t_xab[:], xab[:])
            t_wconv = static.tile([128, 6, 27], BF16)
            nc.sync.dma_start(t_wconv[:], wconv[:])
            t_wmain = static.tile([128, TAPS, 128], BF16)
            nc.sync.dma_start(t_wmain[:], wmain[:])
            t_ck = static.tile([128, 27], F32)
            nc.sync.dma_start(t_ck[:], ck[:])
            t_chv = static.tile([128, Hs], F32)
            nc.sync.dma_start(t_chv[:], chv[:])
            t_pvec = static.tile([128, 1], F32)
            nc.sync.dma_start(t_pvec[:], pvec[:])
            # identity for PE transpose (host-provided)
            t_ident = static.tile([128, 128], BF16)
            nc.sync.dma_start(t_ident[:], ident[:])

            x2ap = x2[:]
            x2ov = bass.AP(tensor=x2ap.tensor, offset=x2ap.offset,
                           ap=[[128, Hp * Wp], [1, ELEM]])
            KH = TAPS * HB

            def prep(hh):
                """conv -> fields -> wrapped idx for chunk hh."""
                h0 = hh * HB
                # ---- conv: offsets+modulator, pos-major [128(w), h, 27] ----
                t_off = offp.tile([128, HB, 27], BF16, tag="off")
                for h in range(HB):
                    hg = h0 + h
                    ps = pconv.tile([128, 27], F32, tag="pc")
                    for j, grp in enumerate(GROUPS):
                        kdim = 64 if len(grp) == 1 else 128
                        o = _conv_off(grp[0], hg)
                        nc.tensor.matmul(
                            ps[:], t_xab[:kdim, o:o + W],
                            t_wconv[:kdim, j, :],
                            start=(j == 0), stop=(j == len(GROUPS) - 1))
                    nc.scalar.copy(t_off[:, h, :], ps[:])

                # ---- fields ----
                # F = off + ck[k] (+ h for y cols, + p for x cols)
                t_F = fld.tile([128, HB, 27], F32, tag="F")
                nc.vector.tensor_tensor(
                    t_F[:], t_off[:],
                    bass.AP(tensor=t_ck[:].tensor, offset=t_ck[:].offset,
                            ap=[list(t_ck[:].ap[0]), [0, HB], [1, 27]]),
                    ALU.add)
                chs = t_chv[:, h0:h0 + HB]
                nc.vector.tensor_tensor(
                    t_F[:, :, 0:9], t_F[:, :, 0:9],
                    bass.AP(tensor=chs.tensor, offset=chs.offset,
                            ap=[list(chs.ap[0]), [1, HB], [0, 9]]),
                    ALU.add)
                nc.vector.tensor_scalar(t_F[:, :, 9:18], t_F[:, :, 9:18],
                                        t_pvec[:], None, ALU.add)
                t_i32 = fld.tile([128, HB, 18], I32, tag="i32")
                nc.vector.tensor_scalar(t_i32[:], t_F[:, :, 0:18], _FLOOR_BIAS,
                                        None, ALU.add)
                t_fl = fld.tile([128, HB, 18], F32, tag="fl")
                nc.vector.tensor_copy(t_fl[:], t_i32[:])
                t_fr = fld.tile([128, HB, 18], F32, tag="fr")
                nc.vector.tensor_tensor(t_fr[:], t_F[:, :, 0:18], t_fl[:],
                                        ALU.subtract)
                ty = t_fr[:, :, 0:9]
                tx = t_fr[:, :, 9:18]
                t_mask = fld.tile([128, HB, 9], F32, tag="mask")
                nc.scalar.activation(t_mask[:], t_F[:, :, 18:27], ACTF.Sigmoid)
                t_w11 = fld.tile([128, HB, 9], F32, tag="w11")
                nc.vector.tensor_tensor(t_w11[:], ty, tx, ALU.mult)
                t_w01 = fld.tile([128, HB, 9], F32, tag="w01")
                nc.vector.tensor_tensor(t_w01[:], tx, t_w11[:], ALU.subtract)
                t_w10 = fld.tile([128, HB, 9], F32, tag="w10")
                nc.vector.tensor_tensor(t_w10[:], ty, t_w11[:], ALU.subtract)
                t_omty = fld.tile([128, HB, 9], F32, tag="omty")
                nc.vector.tensor_scalar(t_omty[:], ty, -1.0, 1.0,
                                        ALU.mult, ALU.add)
                t_w00 = fld.tile([128, HB, 9], F32, tag="w00")
                nc.vector.tensor_tensor(t_w00[:], t_omty[:], t_w01[:],
                                        ALU.subtract)
                # wcomb[p, h, k, xc, yc] bf16, mask folded
                t_wc = fld.tile([128, HB, TAPS, 2, 2], BF16, tag="wc", bufs=3)
                for (xc, yc, tw) in ((0, 0, t_w00), (0, 1, t_w10),
                                     (1, 0, t_w01), (1, 1, t_w11)):
                    nc.vector.tensor_tensor(t_wc[:, :, :, xc, yc], tw[:],
                                            t_mask[:], ALU.mult)
                # linear idx = y0*Wp + x0 (f32 exact) -> int16
                t_lin = fld.tile([128, HB, 9], F32, tag="lin")
                nc.vector.tensor_scalar(t_lin[:], t_fl[:, :, 0:9], float(Wp),
                                        None, ALU.mult)
                nc.vector.tensor_tensor(t_lin[:], t_lin[:], t_fl[:, :, 9:18],
                                        ALU.add)
                # k-major idx tile [128, 9(k), HB(h)]
                t_i16 = fld.tile([128, TAPS, HB], I16, tag="i16")
                nc.vector.tensor_copy(
                    t_i16[:].rearrange("p k h -> p h k"), t_lin[:])

                # ---- idx redistribution to wrapped+replicated layout ----
                # Contiguous DRAM roundtrip (576B descriptors), then the
                # (h,q) interleave runs on DVE where it is free.
                # scratch addr = p*KH + k*HB + h  (per-partition contiguous)
                base = idxs_dram[hh]
                wr_dst = bass.AP(tensor=base.tensor, offset=base.offset,
                                 ap=[[KH, 128], [1, KH]])
                nc.sync.dma_start(wr_dst, t_i16[:])
                # t_idxq[16g+r, (q, k, h)] = scratch[(16q+r)*KH + k*HB + h]
                t_idxq = idxp.tile([128, 8, KH], I16, tag="ixq")
                rd_src = bass.AP(tensor=base.tensor, offset=base.offset,
                                 ap=[[KH, 16], [16 * KH, 8], [1, KH]])
                for g in range(8):
                    nc.sync.dma_start(t_idxq[16 * g:16 * (g + 1)], rd_src)
                # wrapped[16g+r, k, (8h+q)] = idx16[p=16q+r, k, h]
                t_idxw = idxp.tile([128, TAPS, JP], I16, tag="ix")
                ixq = t_idxq[:]
                ow = t_idxw[:]
                in_view = bass.AP(tensor=ixq.tensor, offset=ixq.offset,
                                  ap=[list(ixq.ap[0]), [HB, TAPS], [1, HB],
                                      [KH, 8]])
                out_view = bass.AP(tensor=ow.tensor, offset=ow.offset,
                                   ap=[list(ow.ap[0]), [JP, TAPS], [8, HB],
                                       [1, 8]])
                nc.vector.tensor_copy(out_view, in_view)
                return t_idxw, t_wc

            def consume(hh, t_idxw, t_wc):
                """gather -> combine -> transpose -> einsum for chunk hh."""
                t_st = [stp.tile([128, HB, 128], BF16, tag=f"st{j}",
                                 name=f"st{j}")
                        for j in range(TAPS)]
                for j, grp in enumerate(GROUPS):
                    nk = len(grp)
                    t_v = vp.tile([128, 2, HB, ELEM], BF16, tag="v", name="v")
                    for t, k in enumerate(grp):
                        nc.gpsimd.dma_gather(
                            t_v[:, t], x2ov, t_idxw[:, k, :],
                            NPOS_HB, NPOS_HB, ELEM, elem_step=128,
                            single_packet=False,
                            queue_num=gather_seq[0] % 4)
                        gather_seq[0] += 1
                    # combine: U = V * broadcast(wc), in place.
                    # per (tap, xc): ISA allows only 3 free dims
                    wc = t_wc[:]
                    for t, k in enumerate(grp):
                        for xc in range(2):
                            vv = t_v[:, t, :, xc * 128:(xc + 1) * 128]
                            wv = bass.AP(
                                tensor=wc.tensor,
                                offset=wc.offset + k * 4 + xc * 2,
                                ap=[list(wc.ap[0]), [TAPS * 4, HB],
                                    [0, 64], [1, 2]])
                            nc.vector.tensor_tensor(vv, vv, wv, ALU.mult)
                    # transpose into psum; rows become (c, yc); the x-corner
                    # sum rides the PSUM accumulation (2 matmuls per block);
                    # y-corner sum happens in einsum (duplicated W rows)
                    vap = t_v[:, 0:nk]
                    for t, k in enumerate(grp):
                        for g in range(HB // 8):
                            pt = ptac.tile([128, 8, 128], F32, tag="pt")
                            for h in range(8):
                                hloc = g * 8 + h
                                for xc in range(2):
                                    uap = bass.AP(
                                        tensor=vap.tensor,
                                        offset=vap.offset + t * HB * ELEM
                                        + hloc * ELEM + xc * 128,
                                        ap=[list(vap.ap[0]), [1, 128]])
                                    nc.tensor.matmul(
                                        pt[:, h, :], uap, t_ident[:],
                                        start=(xc == 0), stop=(xc == 1))
                            nc.scalar.copy(
                                t_st[k][:, g * 8:(g + 1) * 8, :], pt[:])

                # ---- einsum: out[o, pos] += wmain_k^T @ S_T_k ----
                for hg in range(HB // 8):
                    po = pout.tile([128, 1024], F32, tag="po")
                    for j in range(TAPS):
                        stv = t_st[j][:].rearrange("p h w -> p (h w)")
                        for nn in range(2):
                            lo = hg * 1024 + nn * 512
                            nc.tensor.matmul(
                                po[:, nn * 512:(nn + 1) * 512],
                                t_wmain[:, j, :], stv[:, lo:lo + 512],
                                start=(j == 0), stop=(j == TAPS - 1))
                    t_out = outp.tile([128, 1024], F32, tag="out")
                    nc.scalar.copy(t_out[:], po[:])
                    base_o = (hh * (HB // 8) + hg) * 1024
                    nc.sync.dma_start(
                        out[:, base_o:base_o + 1024], t_out[:])

            # software pipeline: idx prep runs 2 chunks ahead of consumption
            ready = {}
            for hh in range(min(2, NCH)):
                ready[hh] = prep(hh)
            for hh in range(NCH):
                consume(hh, *ready.pop(hh))
                if hh + 2 < NCH:
                    ready[hh + 2] = prep(hh + 2)

    nc.finalize()
    return nc


# ---------------- host side ----------------

def prep_core_inputs(x, w_off, b_off, w_mod, b_mod, w_reg, s):
    """Build device input dict for shard s (image s//2, rows 64*(s%2)+)."""
    b, half = divmod(s, 2)
    r0 = half * Hs
    xb = np.asarray(x[b], dtype=np.float32)           # [C, H, W]

    # conv input, channel-major padded [64, 66, 130] bf16; second partition
    # half is the same data shifted by +1 element (for pair tap groups)
    xcm = np.zeros((Cin, CONV_H, CONV_W), np.float32)
    ylo = r0 - 1
    sylo, syhi = max(ylo, 0), min(ylo + CONV_H, H)
    xcm[:, sylo - ylo:syhi - ylo, 1:1 + W] = xb[:, sylo:syhi, :]
    xf = xcm.reshape(Cin, CONV_FLAT).astype(ml_dtypes.bfloat16)
    xab = np.zeros((128, CONV_FLAT), ml_dtypes.bfloat16)
    xab[:Cin] = xf
    xab[Cin:, :-1] = xf[:, 1:]

    # X2 gather source: [Hp, Wp, Cin, 2(yc)] -> flat [Hp*Wp (+1), 128]
    X2 = np.zeros((Hp, Wp, Cin, 2), np.float32)
    for yc in range(2):
        ylo = r0 - PADY + yc
        sylo, syhi = max(ylo, 0), min(ylo + Hp, H)
        X2[sylo - ylo:syhi - ylo, PADX:PADX + W, :, yc] = \
            xb[:, sylo:syhi, :].transpose(1, 2, 0)
    x2 = np.zeros((Hp * Wp + 1, Cin * 2), ml_dtypes.bfloat16)
    x2[:Hp * Wp] = X2.reshape(Hp * Wp, Cin * 2).astype(ml_dtypes.bfloat16)

    # decomposed consts: ck[*, 27] per-tap, chv[*, h]=h, pvec[p,1]=p
    ck = np.zeros((128, 27), np.float32)
    for k in range(TAPS):
        ky, kx = divmod(k, 3)
        ck[:, k] = ky + (PADY - 1) + float(b_off[2 * k])
        ck[:, 9 + k] = kx + (PADX - 1) + float(b_off[2 * k + 1])
        ck[:, 18 + k] = float(b_mod[k])
    chv = np.broadcast_to(np.arange(Hs, dtype=np.float32)[None, :],
                          (128, Hs)).copy()
    pvec = np.arange(128, dtype=np.float32).reshape(128, 1).copy()

    # conv weights [128, 6, 27]: group rows = [c of k0 | c of k1]
    wconv = np.zeros((128, 6, 27), np.float32)
    for j, grp in enumerate(GROUPS):
        for t, k in enumerate(grp):
            ky, kx = divmod(k, 3)
            for o in range(9):
                wconv[t * 64:(t + 1) * 64, j, o] = w_off[2 * o, :, ky, kx]
                wconv[t * 64:(t + 1) * 64, j, 9 + o] = w_off[2 * o + 1, :, ky, kx]
                wconv[t * 64:(t + 1) * 64, j, 18 + o] = w_mod[o, :, ky, kx]
    wconv = wconv.astype(ml_dtypes.bfloat16)

    # main weights [128, 9, 128]: chunk k rows = (c, yc) duplicated
    wmain = np.zeros((128, TAPS, 128), np.float32)
    for k in range(TAPS):
        ky, kx = divmod(k, 3)
        wt = 2.0 * w_reg[:, :, ky, kx].T        # [c, o]
        wmain[0::2, k, :] = wt
        wmain[1::2, k, :] = wt
    wmain = wmain.astype(ml_dtypes.bfloat16)

    return {"xab": xab, "x2": x2, "ck": ck, "chv": chv, "pvec": pvec,
            "wconv": wconv, "wmain": wmain,
            "ident": np.eye(128, dtype=ml_dtypes.bfloat16)}


_NC_CACHE = {}


def _run(x, w_off, b_off, w_mod, b_mod, w_reg, trace=False, **spmd_kwargs):
    from concourse.bass_utils import run_bass_kernel_spmd
    x = np.asarray(x); w_off = np.asarray(w_off); b_off = np.asarray(b_off)
    w_mod = np.asarray(w_mod); b_mod = np.asarray(b_mod)
    w_reg = np.asarray(w_reg)

    if "nc" not in _NC_CACHE:
        _NC_CACHE["nc"] = build_nc()
    nc = _NC_CACHE["nc"]

    in_maps = [prep_core_inputs(x, w_off, b_off, w_mod, b_mod, w_reg, s)
               for s in range(8)]
    res = run_bass_kernel_spmd(nc, in_maps, core_ids=list(range(8)),
                               trace=trace, **spmd_kwargs)
    results = res.results if hasattr(res, "results") else res

    out = np.zeros((B, Cout, H, W), np.float32)
    for s in range(8):
        b, half = divmod(s, 2)
        r0 = half * Hs
        out[b, :, r0:r0 + Hs, :] = \
            np.asarray(results[s]["out"]).reshape(Cout, Hs, W)
    return out, res


def kernel(x, w_off, b_off, w_mod, b_mod, w_reg):
    out, _ = _run(x, w_off, b_off, w_mod, b_mod, w_reg)
    return out

